# revision 28
# baseline (speedup 1.0000x reference)
"""Trainium2 Bass kernel for nn_CrossAttention (self-attention, B=2, N=4096,
QD=512, 8 heads x 64 dim).

Sharding: 16 (batch, head) pairs across 8 cores -> core c handles batch c//4
and heads {2*(c%4), 2*(c%4)+1}.  Projection weights are column-sliced (Wq/Wk/Wv)
and row-sliced (Wo) per core; each core emits per-head UNNORMALIZED Wo partials
(bf16) plus the softmax denominator rows; the host divides by the denominators
and sums partials across heads and cores (row-parallel Wo => all-reduce on
host at gather).

Device kernel (per core, 2 heads packed on 128 partitions), software-pipelined
emission (engines execute in FIFO order, so emission order is schedule order).
The scalar engine's exp is the hard floor (256 ACTIVATEs x ~1.03us = 264us
busy); everything else is scheduled to hide under it:
  - x DMA'd in 8 n-chunks (first chunk split and ordered first on the DGE
    ring); k/v projections + V' PE-transposes stream through i_slice 0's
    loop, emitted BEFORE each QK so the in-order PE queue never idles at a
    blocked QK; dummy identity transposes warm the PE p-state during the
    initial DMA wait and hold it at full clock through the drain.
  - per j-tile: row-tiled QK^T pair (K=64 heads at row groups 0/64) -> S^T
    [128j, 512i] per head in one 2-bank PSUM group; one exp ACTIVATE over
    [128, 1024] (scale fused, no max subtraction; |S| <~ 1.5); AV matmuls lag
    by LAG j-steps, with the last NDRAIN AVs of each slice emitted into the
    next slice's first j-iterations so the PE never runs a long AV-only tail.
  - V' has a ones column (65th) so softmax denominators fall out of the AV
    matmul (row 64).  Epilogue per slice: av psum rows 0:63 -> lh bf16
    (unnormalized), row 64 -> den_sb -> DMA; per-head Wo matmuls (K=64, so
    head partials stay separate) into alternating wop psums; copies to bf16
    wos tiles; DMA.  No reciprocal / partition_broadcast / normalize on
    device -- the host computes out0/den0 + out1/den1.
  - final-slice drain: Wo psums borrow the freed st banks for a deeper
    pipeline and copies alternate scalar/vector (the scalar engine is idle
    after the last EXP); copies stage into contiguous wosl tiles flushed by
    one large DMA per head on two DGE queues, replacing eight
    issue-serialized dma_starts.

Rejected experiments (measured): fp8e4 DoubleRow QK^T ran at the SAME
cycles/column as bf16 on HW (cost model's 0.5 cycles/row does not
materialize) and pushed rel err to 1.9e-2; transposed-AV and j-outer AV
layouts lose to LDWEIGHTS overhead; gpsimd-issued tail DMAs were slower
than the sync ring.
"""

import sys

sys.path.insert(0, "/opt/trn_rl_repo")

import numpy as np
import ml_dtypes

import concourse.bass as bass
import concourse.mybir as mybir
from concourse import bacc
from concourse.tile import TileContext
from concourse.bass_utils import run_bass_kernel_spmd
from concourse.masks import make_identity

B, N, QD = 2, 4096, 512
HEADS, DIM_HEAD = 8, 64
INNER = HEADS * DIM_HEAD
SCALE = DIM_HEAD**-0.5

NCORES = 8
HPC = 2  # heads per core
D2 = HPC * DIM_HEAD  # 128
KT = 4  # k tiles of 128 over QD=512
ISL = 512  # i slice
NI = N // ISL  # 8
JTL = 128  # j tile
NJ = N // JTL  # 32
LAG = 6  # AV matmuls trail QK/exp by this many j-steps
NDRAIN = LAG  # last NDRAIN AVs of a slice drain into the next slice's j=0..2

F32 = mybir.dt.float32
BF16 = mybir.dt.bfloat16
BFNP = ml_dtypes.bfloat16
EXP = mybir.ActivationFunctionType.Exp


def build_program():
    nc = bacc.Bacc("TRN2", target_bir_lowering=False, debug=False,
                   num_devices=NCORES)

    xT = nc.dram_tensor("xT", [QD, N], BF16, kind="ExternalInput").ap()
    wq = nc.dram_tensor("wq", [QD, D2], BF16, kind="ExternalInput").ap()
    wk = nc.dram_tensor("wk", [QD, D2], BF16, kind="ExternalInput").ap()
    wv = nc.dram_tensor("wv", [QD, D2], BF16, kind="ExternalInput").ap()
    wo = nc.dram_tensor("wo", [D2, QD], BF16, kind="ExternalInput").ap()
    out0 = nc.dram_tensor("out0", [N, QD], BF16, kind="ExternalOutput").ap()
    out1 = nc.dram_tensor("out1", [N, QD], BF16, kind="ExternalOutput").ap()
    dens = nc.dram_tensor("dens", [HPC, N], F32, kind="ExternalOutput").ap()

    with TileContext(nc) as tc:
        with tc.tile_pool(name="persist", bufs=1) as pp, \
             tc.tile_pool(name="st_ps", bufs=2, space="PSUM") as st_ps, \
             tc.tile_pool(name="av_ps", bufs=1, space="PSUM") as av_ps, \
             tc.tile_pool(name="aux_ps", bufs=2, space="PSUM") as aux_ps, \
             tc.tile_pool(name="p_sb", bufs=10) as p_sb, \
             tc.tile_pool(name="n_sb", bufs=2) as n_sb:
            x_sb = pp.tile([128, KT, N], BF16)
            wq_sb = pp.tile([128, KT, D2], BF16)
            wk_sb = pp.tile([128, KT, D2], BF16)
            wv_sb = pp.tile([128, KT, D2], BF16)
            wo_sb = pp.tile([128, QD], BF16)
            ident = pp.tile([128, 128], BF16)
            qT = pp.tile([128, N], BF16)
            kT = pp.tile([128, N], BF16)
            vT = pp.tile([128, N], BF16)
            v0p = pp.tile([128, NJ, DIM_HEAD + 1], BF16)
            v1p = pp.tile([128, NJ, DIM_HEAD + 1], BF16)
            wosl0 = pp.tile([128, 4, QD], BF16)
            wosl1 = pp.tile([128, 4, QD], BF16)

            # DMA order matters: the first projection needs wk + x chunk 0,
            # so issue those first on the (in-order) sync DGE ring.
            xTr = xT.rearrange("(k p) n -> p k n", p=128)
            nc.sync.dma_start(out=wk_sb[:], in_=wk.rearrange("(k p) m -> p k m", p=128))
            for k0 in range(KT):
                nc.sync.dma_start(out=x_sb[:, k0:k0 + 1, 0:ISL],
                                  in_=xTr[:, k0:k0 + 1, 0:ISL])
            nc.sync.dma_start(out=wq_sb[:], in_=wq.rearrange("(k p) m -> p k m", p=128))
            nc.sync.dma_start(out=wv_sb[:], in_=wv.rearrange("(k p) m -> p k m", p=128))
            nc.sync.dma_start(out=wo_sb[:], in_=wo[:])
            for s in range(1, NI):
                ssl = slice(s * ISL, (s + 1) * ISL)
                nc.sync.dma_start(out=x_sb[:, :, ssl], in_=xTr[:, :, ssl])
            make_identity(nc, ident[:])
            nc.gpsimd.memset(v0p[:, :, DIM_HEAD], 1.0)
            nc.gpsimd.memset(v1p[:, :, DIM_HEAD], 1.0)
            # Warm the PE (p-state ramps over ~3us of continuous work) with
            # dummy transposes of the identity while the x DMA is in flight.
            for w in range(26):
                wt = aux_ps.tile([128, 128], BF16, tag="aux", name="wt")
                nc.tensor.transpose(wt[:], ident[:], ident[:])

            def proj(w_sb, dst, s):
                """dst[:, s*ISL:(s+1)*ISL] = (W^T @ x^T) slice, via aux psum."""
                ssl = slice(s * ISL, (s + 1) * ISL)
                ps = aux_ps.tile([128, ISL], F32, tag="aux", name="ps")
                for k in range(KT):
                    nc.tensor.matmul(ps[:], w_sb[:, k, :], x_sb[:, k, ssl],
                                     start=(k == 0), stop=(k == KT - 1))
                nc.vector.tensor_copy(out=dst[:, ssl], in_=ps[:])

            def transp(j):
                """V'[j] tiles from vT via PE transpose (both heads)."""
                tp = aux_ps.tile([128, 128], BF16, tag="aux")
                nc.tensor.transpose(tp[:], vT[:, j * JTL:(j + 1) * JTL], ident[:])
                nc.vector.tensor_copy(out=v0p[:, j, 0:DIM_HEAD], in_=tp[:, 0:DIM_HEAD])
                nc.vector.tensor_copy(out=v1p[:, j, 0:DIM_HEAD], in_=tp[:, DIM_HEAD:D2])

            # per-slice state carried across the pipeline
            states = {}

            def emit_av(i_a, ja):
                e = states[i_a]
                pt = e["pts"].pop(ja)
                nc.tensor.matmul(e["av0"][:], v0p[:, ja, :], pt[:, 0:ISL],
                                 start=(ja == 0), stop=(ja == NJ - 1))
                nc.tensor.matmul(e["av1"][:], v1p[:, ja, :], pt[:, ISL:2 * ISL],
                                 start=(ja == 0), stop=(ja == NJ - 1))

            def emit_copies(i_prev, tail=False):
                """av psum -> lh (unnormalized bf16) + den rows -> sbuf."""
                e = states[i_prev]
                e["lh"] = n_sb.tile([128, ISL], BF16, tag="lh", name="lh", bufs=3)
                e["den0"] = n_sb.tile([1, ISL], F32, tag="den0", name="den0")
                e["den1"] = n_sb.tile([1, ISL], F32, tag="den1", name="den1")
                if tail:
                    # scalar engine is idle after the last EXP, but cannot
                    # shift partition bases -- only the head-0 copy goes there
                    nc.scalar.copy(out=e["lh"][0:64, :], in_=e["av0"][0:DIM_HEAD, :])
                else:
                    nc.vector.tensor_copy(out=e["lh"][0:64, :], in_=e["av0"][0:DIM_HEAD, :])
                nc.vector.tensor_copy(out=e["lh"][64:128, :], in_=e["av1"][0:DIM_HEAD, :])
                nc.vector.tensor_copy(out=e["den0"][:], in_=e["av0"][DIM_HEAD:DIM_HEAD + 1, :])
                nc.vector.tensor_copy(out=e["den1"][:], in_=e["av1"][DIM_HEAD:DIM_HEAD + 1, :])
                isl = slice(i_prev * ISL, (i_prev + 1) * ISL)
                nc.sync.dma_start(out=dens[0:1, isl], in_=e["den0"][:])
                nc.sync.dma_start(out=dens[1:2, isl], in_=e["den1"][:])

            def emit_wo(i_prev, b, tail=False):
                """Per-head Wo matmuls for i-block b of slice i_prev + DMA.
                Tail blocks stage into contiguous wosl tiles; one big DMA per
                head at b==3 replaces 8 issue-serialized dma_starts."""
                e = states[i_prev]
                bsl = slice(b * 128, (b + 1) * 128)
                r0 = i_prev * ISL + b * 128
                pool0 = st_ps if tail else aux_ps
                tag0 = "st" if tail else "aux"
                wop0 = pool0.tile([128, QD], F32, tag=tag0, name="wop0")
                nc.tensor.matmul(wop0[:], e["lh"][0:64, bsl], wo_sb[0:64, :],
                                 start=True, stop=True)
                if tail:
                    nc.scalar.copy(out=wosl0[:, b, :], in_=wop0[:])
                else:
                    wos0 = n_sb.tile([128, QD], BF16, tag="wos0")
                    nc.vector.tensor_copy(out=wos0[:], in_=wop0[:])
                    nc.sync.dma_start(out=out0[r0:r0 + 128, :], in_=wos0[:])
                wop1 = aux_ps.tile([128, QD], F32, tag="aux", name="wop1")
                nc.tensor.matmul(wop1[:], e["lh"][64:128, bsl], wo_sb[64:128, :],
                                 start=True, stop=True)
                if tail:
                    nc.vector.tensor_copy(out=wosl1[:, b, :], in_=wop1[:])
                else:
                    wos1 = n_sb.tile([128, QD], BF16, tag="wos1")
                    nc.vector.tensor_copy(out=wos1[:], in_=wop1[:])
                    nc.sync.dma_start(out=out1[r0:r0 + 128, :], in_=wos1[:])
                if tail:
                    rsl = i_prev * ISL + b * 128
                    nc.sync.dma_start(out=out0[rsl:rsl + 128, :],
                                      in_=wosl0[:, b, :])
                    nc.scalar.dma_start(out=out1[rsl:rsl + 128, :],
                                        in_=wosl1[:, b, :])

            # warm up slice 0 of k/q projections before the attention loop
            # (v projection + V' transposes are deferred into early j slots).
            # Copies are split across scalar+vector and ordered so QK(0,0)'s
            # exact operands (kT j-tile 0, qT head rows) land first.
            ps = aux_ps.tile([128, ISL], F32, tag="aux", name="ps")
            for k in range(KT):
                nc.tensor.matmul(ps[:], wk_sb[:, k, :], x_sb[:, k, 0:ISL],
                                 start=(k == 0), stop=(k == KT - 1))
            nc.vector.tensor_copy(out=kT[:, 0:JTL], in_=ps[:, 0:JTL])
            nc.vector.tensor_copy(out=kT[:, JTL:ISL], in_=ps[:, JTL:ISL])
            ps = aux_ps.tile([128, ISL], F32, tag="aux", name="ps")
            for k in range(KT):
                nc.tensor.matmul(ps[:], wq_sb[:, k, :], x_sb[:, k, 0:ISL],
                                 start=(k == 0), stop=(k == KT - 1))
            nc.vector.tensor_copy(out=qT[0:64, 0:ISL], in_=ps[0:64, :])
            nc.vector.tensor_copy(out=qT[64:128, 0:ISL], in_=ps[64:128, :])
            proj(wv_sb, vT, 0)

            for i in range(NI):
                isl = slice(i * ISL, (i + 1) * ISL)
                states[i] = {
                    "av0": av_ps.tile([DIM_HEAD + 1, ISL], F32, tag="av0",
                                      name="av0"),
                    "av1": av_ps.tile([DIM_HEAD + 1, ISL], F32, tag="av1",
                                      name="av1"),
                    "pts": {},
                }
                for j in range(NJ):
                    jsl = slice(j * JTL, (j + 1) * JTL)
                    # prologue interleaves (i == 0) go BEFORE the QK: the PE
                    # queue is in-order, so a blocked QK (waiting on an EXP to
                    # free an st buf) must not trap ready proj work behind it
                    if i == 0 and j == 0:
                        for jj in range(4):
                            transp(jj)
                    elif i == 0 and j % 4 in (1, 2, 3):
                        s = j // 4 + 1
                        if s < NI:
                            if j % 4 == 1:
                                proj(wk_sb, kT, s)
                            elif j % 4 == 2:
                                proj(wv_sb, vT, s)
                            else:
                                for jj in range(4 * s, 4 * s + 4):
                                    transp(jj)
                    st = st_ps.tile([128, 2 * ISL], F32, tag="st")
                    nc.tensor.matmul(st[:, 0:ISL], kT[0:64, jsl], qT[0:64, isl],
                                     start=True, stop=True)
                    nc.tensor.matmul(st[:, ISL:2 * ISL], kT[64:128, jsl],
                                     qT[64:128, isl], start=True, stop=True)
                    pt = p_sb.tile([128, 2 * ISL], BF16, tag="pt")
                    nc.scalar.activation(pt[:], st[:], EXP, scale=SCALE)
                    states[i]["pts"][j] = pt
                    # drain AVs of the previous slice (two per step, j=0..2)
                    if i > 0 and j < 3:
                        emit_av(i - 1, NJ - NDRAIN + 2 * j)
                        emit_av(i - 1, NJ - NDRAIN + 2 * j + 1)
                    # previous slice fully accumulated: pull av psum to sbuf
                    if i > 0 and j == 3:
                        emit_copies(i - 1)
                    # this slice's AVs, LAG j-steps behind
                    if j >= LAG:
                        emit_av(i, j - LAG)
                    # Wo + store for the previous slice
                    if i > 0 and j in (8, 10, 12, 14):
                        emit_wo(i - 1, (j - 8) // 2)
                    # next slice's q projection
                    if j == 16 and i + 1 < NI:
                        proj(wq_sb, qT, i + 1)
                    # last slice: no next-slice QK work exists, so feed the PE
                    # dummy transposes to hold its p-state at full clock
                    if i == NI - 1 and j >= 28:
                        for w in range(3):
                            wt2 = aux_ps.tile([128, 128], BF16, tag="aux",
                                              name="wt2")
                            nc.tensor.transpose(wt2[:], ident[:], ident[:])

            # final drain: slice NI-1's last AVs + epilogue
            last = NI - 1
            for ja in range(NJ - NDRAIN, NJ):
                emit_av(last, ja)
                wt3 = aux_ps.tile([128, 128], BF16, tag="aux", name="wt3")
                nc.tensor.transpose(wt3[:], ident[:], ident[:])
            emit_copies(last, tail=True)
            for b in range(4):
                emit_wo(last, b, tail=True)

    nc.compile()
    return nc


_NC = None


def _get_program():
    global _NC
    if _NC is None:
        _NC = build_program()
    return _NC


def _in_maps(x, Wq, Wk, Wv, Wo):
    in_maps = []
    for c in range(NCORES):
        b, m = divmod(c, NCORES // B)
        cs = slice(m * D2, (m + 1) * D2)
        in_maps.append({
            "xT": np.ascontiguousarray(x[b].T).astype(BFNP),
            "wq": np.ascontiguousarray(Wq[:, cs]).astype(BFNP),
            "wk": np.ascontiguousarray(Wk[:, cs]).astype(BFNP),
            "wv": np.ascontiguousarray(Wv[:, cs]).astype(BFNP),
            "wo": np.ascontiguousarray(Wo[cs, :]).astype(BFNP),
        })
    return in_maps


def _gather(res, bo):
    out = np.zeros((B, N, QD), dtype=np.float32)
    for c in range(NCORES):
        b = c // (NCORES // B)
        r = res.results[c]
        d = r["dens"]
        out[b] += (r["out0"].astype(np.float32) / d[0][:, None]
                   + r["out1"].astype(np.float32) / d[1][:, None])
    out += bo[None, None, :]
    return out


def kernel(x, Wq, Wk, Wv, Wo, bo):
    x = np.asarray(x, dtype=np.float32)
    Wq = np.asarray(Wq, dtype=np.float32)
    Wk = np.asarray(Wk, dtype=np.float32)
    Wv = np.asarray(Wv, dtype=np.float32)
    Wo = np.asarray(Wo, dtype=np.float32)
    bo = np.asarray(bo, dtype=np.float32)

    nc = _get_program()
    res = run_bass_kernel_spmd(nc, _in_maps(x, Wq, Wk, Wv, Wo),
                               core_ids=list(range(NCORES)))
    return _gather(res, bo)
